# revision 23
# baseline (speedup 1.0000x reference)
"""Trainium2 Bass kernel for nn_DenseRelationModule.

Computes, for full inputs (B=2, N=512):
  emb = l2norm(object_embeddings); lang = l2norm(utterance_features)
  hi[n]  = emb[n] @ Wi - geom[n] @ Wg          (i-terms)
  hj[n]  = emb[n] @ Wj + geom[n] @ Wg          (j-terms)
  hl     = lang @ Wl + b1
  s[i,j] = W2 . gelu(hi[i] + hj[j] + hl) + b2
  P      = softmax_j(s)
  relation_scores[i]  = sum_j P[i,j] * s[i,j]
  relation_context[i] = sum_j P[i,j] * emb[j]

Sharding: 8 cores = 2 batches x 4 i-blocks of 128 rows. Each core computes
its i-block of the N^2 pair grid; host concatenates shard outputs.
"""

import sys

sys.path.insert(0, "/opt/trn_rl_repo")

import numpy as np

OBJ_D, LANG_D, GEO_D, HID = 320, 256, 6, 256
N = 512
NI = 128          # i-rows per core
IB = 8            # i's per gelu mega-instruction
NB = NI // IB     # batches per core
PAIR_D = 2 * OBJ_D + GEO_D + LANG_D

_CACHE = {}
TRACE = False
TRACE_KW = {}
LAST_RESULT = None


def _pstride(ap, step, count):
    """Partition-strided view: rows 0, step, 2*step, ... of a 2D AP."""
    import concourse.bass as bass

    a = ap[:, :]
    newap = [[int(a.ap[0][0]) * step, count]] + [list(map(int, d)) for d in a.ap[1:]]
    return bass.AP(tensor=a.tensor, offset=a.offset, ap=newap)


def _build():
    import concourse.mybir as mybir
    import concourse.tile as tile
    import concourse.bacc as bacc
    from concourse.masks import make_identity
    from concourse.tile import add_dep_helper

    def _chain(insts):
        # PSUM accumulation groups are order-sensitive (start=True clears the
        # bank's has_written bits); Tile's scheduler reorders by readiness, so
        # pin the program order explicitly.
        for prev, nxt in zip(insts[:-1], insts[1:]):
            add_dep_helper(nxt.ins, prev.ins, reason="psum accum order")

    f32 = mybir.dt.float32
    bf16 = mybir.dt.bfloat16
    AF = mybir.ActivationFunctionType
    AX = mybir.AxisListType

    D, L, G, H = OBJ_D, LANG_D, GEO_D, HID
    DT = [128, 128, 64]  # d (=320) partition tiles

    nc = bacc.Bacc("TRN2", target_bir_lowering=False, debug=False, num_devices=8)

    emb = nc.dram_tensor("emb", [N, D], f32, kind="ExternalInput").ap()
    embT = nc.dram_tensor("embT", [D, N], f32, kind="ExternalInput").ap()
    embTi = nc.dram_tensor("embTi", [D, NI], f32, kind="ExternalInput").ap()
    embi = nc.dram_tensor("embi", [NI, D], f32, kind="ExternalInput").ap()
    geomT = nc.dram_tensor("geomT", [G, N], f32, kind="ExternalInput").ap()
    geomTi = nc.dram_tensor("geomTi", [G, NI], f32, kind="ExternalInput").ap()
    lang = nc.dram_tensor("lang", [1, L], f32, kind="ExternalInput").ap()
    W1 = nc.dram_tensor("W1", [PAIR_D, H], f32, kind="ExternalInput").ap()
    b1 = nc.dram_tensor("b1", [1, H], f32, kind="ExternalInput").ap()
    W2 = nc.dram_tensor("W2", [H, 1], f32, kind="ExternalInput").ap()
    b2 = nc.dram_tensor("b2", [1, 1], f32, kind="ExternalInput").ap()
    out_s = nc.dram_tensor("out_s", [NI, 1], f32, kind="ExternalOutput").ap()
    out_c = nc.dram_tensor("out_c", [NI, D], f32, kind="ExternalOutput").ap()
    dr_r = nc.dram_tensor("dr_r", [N], f32).ap()
    dr_lang = nc.dram_tensor("dr_lang", [L], f32).ap()
    dr_hl = nc.dram_tensor("dr_hl", [H], f32).ap()

    with tile.TileContext(nc) as tc:
      with tc.tile_pool(name="sg", bufs=1) as sg:
        # persistent tiles
        emb_sb = sg.tile([128, 4, D], f32)
        embi_sb = sg.tile([128, D], f32)
        embT_bf = sg.tile([128, 3, N], bf16)
        embTi_bf = sg.tile([128, 3, NI], bf16)
        geomT_bf = sg.tile([G, N], bf16)
        geomTi_bf = sg.tile([G, NI], bf16)
        lang_row = sg.tile([1, L], f32)
        Wij_bf = sg.tile([128, 12, 128], bf16)  # (w, dt, ht) -> w*6+dt*2+ht
        Wg_bf = sg.tile([G, H], bf16)
        Wl_sb = sg.tile([128, 2, H], f32)
        W2_bf = sg.tile([128, 2], bf16)
        b1_row = sg.tile([1, H], f32)
        b2_bc = sg.tile([128, 1], f32)
        r_col = sg.tile([128, 5], f32)
        emb_bf = sg.tile([128, 4, D], bf16)
        hjL_sb = sg.tile([128, 2, N], bf16)
        hi_sb = sg.tile([128, 2, NI], f32)
        bias_cols = sg.tile([128, 2], f32)
        scores_sb = sg.tile([128, N], f32)

        # ================= preprocessing =================
        with tc.tile_pool(name="pre", bufs=2) as pre, \
             tc.tile_pool(name="pps", bufs=1, space="PSUM") as pps:
            # ---------- input loads ----------
            nc.sync.dma_start(out=emb_sb, in_=emb.rearrange("(t p) d -> p t d", p=128))
            nc.sync.dma_start(out=embi_sb, in_=embi)
            for dt in range(3):
                dsz, d0 = DT[dt], 128 * dt
                nc.gpsimd.dma_start(out=embT_bf[:dsz, dt], in_=embT[d0:d0 + dsz, :])
                nc.gpsimd.dma_start(out=embTi_bf[:dsz, dt], in_=embTi[d0:d0 + dsz, :])
            nc.gpsimd.dma_start(out=geomT_bf, in_=geomT)
            nc.gpsimd.dma_start(out=geomTi_bf, in_=geomTi)
            nc.sync.dma_start(out=lang_row, in_=lang)
            for w in range(2):
                for dt in range(3):
                    dsz = DT[dt]
                    for ht in range(2):
                        nc.gpsimd.dma_start(
                            out=Wij_bf[:dsz, w * 6 + dt * 2 + ht],
                            in_=W1[w * D + 128 * dt: w * D + 128 * dt + dsz,
                                   128 * ht: 128 * (ht + 1)])
            nc.gpsimd.dma_start(out=Wg_bf, in_=W1[2 * D: 2 * D + G, :])
            for kt in range(2):
                r0 = 2 * D + G + 128 * kt
                nc.sync.dma_start(out=Wl_sb[:, kt], in_=W1[r0:r0 + 128, :])
            for ht in range(2):
                nc.gpsimd.dma_start(out=W2_bf[:, ht:ht + 1],
                                    in_=W2[128 * ht: 128 * (ht + 1), :])
            nc.sync.dma_start(out=b1_row, in_=b1)
            nc.gpsimd.dma_start(out=b2_bc, in_=b2.to_broadcast([128, 1]))

            # ---------- l2 norms ----------
            s_col = pre.tile([128, 5], f32, bufs=1)
            for nt in range(4):
                sq = pre.tile([128, D], f32, tag="sq")
                nc.vector.tensor_mul(sq, emb_sb[:, nt], emb_sb[:, nt])
                nc.vector.reduce_sum(s_col[:, nt:nt + 1], sq, axis=AX.X)
            sq = pre.tile([128, D], f32, tag="sq")
            nc.vector.tensor_mul(sq, embi_sb, embi_sb)
            nc.vector.reduce_sum(s_col[:, 4:5], sq, axis=AX.X)
            nrm = pre.tile([128, 5], f32, bufs=1)
            nc.scalar.activation(out=nrm, in_=s_col, func=AF.Sqrt)
            r0 = pre.tile([128, 5], f32, bufs=1)
            nc.vector.reciprocal(r0, nrm)
            # Newton step for rsqrt: r = r0*(1.5 - 0.5*s*r0^2); the ACT Sqrt
            # table is low-precision (~2^-16 budget) and dominates output err.
            t1 = pre.tile([128, 5], f32, bufs=1)
            nc.vector.tensor_mul(t1, r0, r0)
            nc.vector.tensor_mul(t1, t1, s_col)
            nc.vector.tensor_scalar(t1, t1, -0.5, 1.5,
                                    op0=mybir.AluOpType.mult,
                                    op1=mybir.AluOpType.add)
            nc.vector.tensor_mul(r_col, r0, t1)

            sqL = pre.tile([1, L], f32, bufs=1)
            nc.vector.tensor_mul(sqL, lang_row, lang_row)
            sL = pre.tile([1, 2], f32, bufs=1)
            nc.vector.reduce_sum(sL[:, 0:1], sqL, axis=AX.X)
            nc.scalar.activation(out=sL[:, 1:2], in_=sL[:, 0:1], func=AF.Sqrt)
            rL0 = pre.tile([1, 1], f32, bufs=1)
            nc.vector.reciprocal(rL0, sL[:, 1:2])
            tL = pre.tile([1, 1], f32, bufs=1)
            nc.vector.tensor_mul(tL, rL0, rL0)
            nc.vector.tensor_mul(tL, tL, sL[:, 0:1])
            nc.vector.tensor_scalar(tL, tL, -0.5, 1.5,
                                    op0=mybir.AluOpType.mult,
                                    op1=mybir.AluOpType.add)
            rL = pre.tile([1, 1], f32, bufs=1)
            nc.vector.tensor_mul(rL, rL0, tL)
            nc.vector.tensor_scalar_mul(lang_row, lang_row, rL)

            # reshape r into rows (cross-partition moves via DRAM bounce)
            nc.sync.dma_start(out=dr_r.rearrange("(c p) -> p c", p=128),
                              in_=r_col[:, 0:4])
            r_row = pre.tile([1, N], f32, bufs=1)
            nc.sync.dma_start(out=r_row, in_=dr_r)
            ri_row = pre.tile([1, NI], f32, bufs=1)
            nc.sync.dma_start(out=ri_row, in_=r_col[:, 4:5])
            nc.sync.dma_start(out=dr_lang, in_=lang_row)
            lang_col = pre.tile([128, 2], f32, bufs=1)
            nc.sync.dma_start(out=lang_col,
                              in_=dr_lang.rearrange("(c p) -> p c", p=128))

            # normalized bf16 emb for the context matmul
            for nt in range(4):
                nc.vector.tensor_scalar_mul(emb_bf[:, nt], emb_sb[:, nt],
                                            r_col[:, nt:nt + 1])

            # ---------- broadcast r across partitions (outer products) ----------
            ones = pre.tile([1, 128], f32, bufs=1)
            nc.vector.memset(ones, 1.0)
            r_bc_ps = pps.tile([128, N], f32, tag="r_bc", bufs=1)
            nc.tensor.matmul(r_bc_ps, ones, r_row)
            r_bc = pre.tile([128, N], f32, bufs=1)
            nc.vector.tensor_copy(r_bc, r_bc_ps)
            ri_bc_ps = pps.tile([128, NI], f32, tag="ri_bc", bufs=1)
            nc.tensor.matmul(ri_bc_ps, ones, ri_row)
            ri_bc = pre.tile([128, NI], f32, bufs=1)
            nc.vector.tensor_copy(ri_bc, ri_bc_ps)

            # ---------- hl + b1 -> per-partition bias columns ----------
            hl_ps = pps.tile([1, H], f32, tag="hl", bufs=1)
            _chain([
                nc.tensor.matmul(hl_ps, lang_col[:, 0:1], Wl_sb[:, 0],
                                 start=True, stop=False),
                nc.tensor.matmul(hl_ps, lang_col[:, 1:2], Wl_sb[:, 1],
                                 start=False, stop=True),
            ])
            hlb1_row = pre.tile([1, H], f32, bufs=1)
            nc.vector.tensor_add(hlb1_row, hl_ps, b1_row)
            nc.sync.dma_start(out=dr_hl, in_=hlb1_row)
            nc.sync.dma_start(out=bias_cols,
                              in_=dr_hl.rearrange("(c p) -> p c", p=128))

            # ---------- hjL[h, j] and hi[h, i] ----------
            for ht in range(2):
                # hj = (norm(e) @ Wj) + geom @ Wg; only the emb part is scaled
                # by r, so keep the geom term in its own psum tile.
                hj_ps = pps.tile([128, N], f32, tag="hj_ps", bufs=1)
                _chain([nc.tensor.matmul(hj_ps, Wij_bf[:DT[dt], 6 + dt * 2 + ht],
                                         embT_bf[:DT[dt], dt],
                                         start=(dt == 0), stop=(dt == 2))
                        for dt in range(3)])
                hjG_ps = pps.tile([128, N], f32, tag="hjG_ps", bufs=1)
                nc.tensor.matmul(hjG_ps, Wg_bf[:, 128 * ht: 128 * (ht + 1)],
                                 geomT_bf)
                hj_f = pre.tile([128, N], f32, tag="hj_f")
                nc.vector.tensor_mul(hj_f, hj_ps, r_bc)
                nc.vector.tensor_add(hjL_sb[:, ht], hj_f, hjG_ps)

                hiW_ps = pps.tile([128, NI], f32, tag="hiW_ps", bufs=1)
                _chain([nc.tensor.matmul(hiW_ps, Wij_bf[:DT[dt], dt * 2 + ht],
                                         embTi_bf[:DT[dt], dt],
                                         start=(dt == 0), stop=(dt == 2))
                        for dt in range(3)])
                hiG_ps = pps.tile([128, NI], f32, tag="hiG_ps", bufs=1)
                nc.tensor.matmul(hiG_ps, Wg_bf[:, 128 * ht: 128 * (ht + 1)],
                                 geomTi_bf)
                hi_f = pre.tile([128, NI], f32, tag="hi_f")
                nc.vector.tensor_mul(hi_f, hiW_ps, ri_bc)
                nc.vector.tensor_sub(hi_sb[:, ht], hi_f, hiG_ps)

        # ================= main loop =================
        with tc.tile_pool(name="mv", bufs=4) as vp, \
             tc.tile_pool(name="mg", bufs=4) as gp, \
             tc.tile_pool(name="mst", bufs=4) as stp, \
             tc.tile_pool(name="mps", bufs=6, space="PSUM") as mps:
            for g in range(NB):
                Gt = []
                for ht in range(2):
                    V = vp.tile([128, IB * N], bf16, tag="V")
                    for k in range(IB):
                        nc.vector.tensor_scalar_add(
                            V[:, k * N:(k + 1) * N], hjL_sb[:, ht],
                            hi_sb[:, ht, g * IB + k: g * IB + k + 1])
                    Gg = gp.tile([128, IB * N], bf16, tag="G")
                    nc.scalar.activation(out=Gg, in_=V, func=AF.Gelu,
                                         bias=bias_cols[:, ht:ht + 1], scale=1.0)
                    Gt.append(Gg)
                for q in range(2):
                    ps = mps.tile([128, N], f32, tag="ps", bufs=6)
                    mms = []
                    for kp in range(4):
                        k = q * 4 + kp
                        for ht in range(2):
                            mms.append(nc.tensor.matmul(
                                ps[32 * kp: 32 * kp + 1, :],
                                W2_bf[:, ht:ht + 1],
                                Gt[ht][:, k * N:(k + 1) * N],
                                start=(ht == 0), stop=(ht == 1),
                                tile_position=(0, 32 * kp)))
                    _chain(mms)
                    st = stp.tile([128, N], f32, tag="st")
                    nc.vector.tensor_scalar_add(st, ps, b2_bc)
                    i0 = g * IB + q * 4
                    nc.sync.dma_start(out=scores_sb[i0:i0 + 4, :],
                                      in_=_pstride(st, 32, 4))

        # ================= softmax / outputs =================
        with tc.tile_pool(name="tl", bufs=1) as tl, \
             tc.tile_pool(name="tps", bufs=2, space="PSUM") as tps:
            E_bf = tl.tile([128, N], bf16)
            nc.scalar.activation(out=E_bf, in_=scores_sb, func=AF.Exp)
            denom = tl.tile([128, 1], f32)
            E_f = tl.tile([128, N], f32)
            nc.scalar.activation(out=E_f, in_=scores_sb, func=AF.Exp,
                                 accum_out=denom)
            rden = tl.tile([128, 1], f32)
            nc.vector.reciprocal(rden, denom)

            id_bf = tl.tile([128, 128], bf16)
            make_identity(nc, id_bf)
            ET_sb = tl.tile([128, 4, 128], bf16)
            for jt in range(4):
                tp = tps.tile([128, 128], bf16, tag="tp")
                nc.tensor.transpose(tp, E_bf[:, 128 * jt: 128 * (jt + 1)], id_bf)
                nc.vector.tensor_copy(ET_sb[:, jt], tp)
            ctx_ps = tps.tile([128, D], f32, tag="ctx", bufs=1)
            _chain([nc.tensor.matmul(ctx_ps, ET_sb[:, jt], emb_bf[:, jt],
                                     start=(jt == 0), stop=(jt == 3))
                    for jt in range(4)])
            ctx_sb = tl.tile([128, D], f32)
            nc.vector.tensor_scalar_mul(ctx_sb, ctx_ps, rden)
            nc.sync.dma_start(out=out_c, in_=ctx_sb)

            tmp = tl.tile([128, N], f32)
            nc.vector.tensor_mul(tmp, E_f, scores_sb)
            rsum = tl.tile([128, 1], f32)
            nc.vector.reduce_sum(rsum, tmp, axis=AX.X)
            rs_sb = tl.tile([128, 1], f32)
            nc.vector.tensor_scalar_mul(rs_sb, rsum, rden)
            nc.sync.dma_start(out=out_s, in_=rs_sb)

    nc.compile()
    return nc


def kernel(object_embeddings, object_geometry, utterance_features, W1, b1, W2, b2):
    from concourse.bass_utils import run_bass_kernel_spmd

    global LAST_RESULT
    if "nc" not in _CACHE:
        _CACHE["nc"] = _build()
    nc = _CACHE["nc"]

    emb = np.asarray(object_embeddings, dtype=np.float32)
    geom = np.asarray(object_geometry, dtype=np.float32)
    lang = np.asarray(utterance_features, dtype=np.float32)
    W1 = np.asarray(W1, dtype=np.float32)
    b1 = np.asarray(b1, dtype=np.float32)
    W2 = np.asarray(W2, dtype=np.float32)
    b2 = np.asarray(b2, dtype=np.float32)
    B = emb.shape[0]

    c = np.ascontiguousarray
    in_maps = []
    for core in range(8):
        b, t = core // 4, core % 4
        i0 = t * NI
        embT = c(emb[b].T)
        geomT = c(geom[b].T)
        in_maps.append({
            "emb": emb[b],
            "embT": embT,
            "embTi": c(embT[:, i0:i0 + NI]),
            "embi": c(emb[b, i0:i0 + NI]),
            "geomT": geomT,
            "geomTi": c(geomT[:, i0:i0 + NI]),
            "lang": lang[b].reshape(1, LANG_D),
            "W1": W1,
            "b1": b1.reshape(1, HID),
            "W2": W2.reshape(HID, 1),
            "b2": b2.reshape(1, 1),
        })

    res = run_bass_kernel_spmd(nc, in_maps, list(range(8)),
                               trace=TRACE, **TRACE_KW)
    LAST_RESULT = res

    rel_scores = np.empty((B, N), np.float32)
    rel_context = np.empty((B, N, OBJ_D), np.float32)
    for core in range(8):
        b, t = core // 4, core % 4
        i0 = t * NI
        rel_scores[b, i0:i0 + NI] = res.results[core]["out_s"][:, 0]
        rel_context[b, i0:i0 + NI] = res.results[core]["out_c"]
    return rel_scores, rel_context
